# revision 1
# baseline (speedup 1.0000x reference)
"""Trainium2 Bass kernel: ISTFT -> Butterworth filtfilt -> STFT (LowpassFilter).

Strategy: the whole reference pipeline is linear. Per batch sample:
  - irfft+synthesis-window  == matmul with a precomputed [1024, 1022] matrix
  - overlap-add             == 8 strided DVE adds into a blocked signal buffer
  - filtfilt                == the IIR impulse response decays below 1e-14 by
                               lag 96 (max pole radius 0.7265), so each pass is
                               exactly (to fp32) a causal/anti-causal FIR of
                               length 128 == block-Toeplitz matmuls with one
                               off-diagonal block, plus rank-1 edge corrections
                               for the odd-extension padding + zi seeding.
  - framing+rfft+fwd-window == matmul with a precomputed [1024, 1024] matrix
Data parallel: batch 64 -> 8 samples per NeuronCore, SPMD on 8 cores.
"""

import numpy as np

W = 1022
HOP = 256
F = 64
ORDER = 5
WN = 0.5
T = HOP * (F - 1) + W  # 17150
KH = 128               # FIR truncation of the IIR impulse response
NBU = 137              # signal blocks per channel in U: [lookback, b0..b135]
S = 8                  # samples (channels) per core

MM_DT = "float32r"     # dtype for the big FFT matmuls: "float32" or "float32r"


# ---------------------------------------------------------------- constants
def _butter_lowpass(order, wn):
    m = np.arange(-order + 1, order, 2)
    p = -np.exp(1j * np.pi * m / (2 * order))
    fs = 2.0
    warped = 2 * fs * np.tan(np.pi * wn / fs)
    p = p * warped
    k = warped ** order
    fs2 = 2 * fs
    pd = (fs2 + p) / (fs2 - p)
    kd = k * np.real(1.0 / np.prod(fs2 - p))
    b = np.real(kd * np.poly(-np.ones(order)))
    a = np.real(np.poly(pd))
    return b / a[0], a / a[0]


def _build_consts():
    B, A = _butter_lowpass(ORDER, WN)
    n = max(len(A), len(B))
    Am0 = np.zeros((n - 1, n - 1))
    Am0[0, :] = -A[1:]
    Am0[1:, :-1] = np.eye(n - 2)
    Am0 = Am0.T
    ZI = np.linalg.solve(np.eye(n - 1) - Am0, B[1:] - A[1:] * B[0])

    b0 = B[0]
    n5 = 5
    Am = np.zeros((n5, n5))
    for i in range(n5):
        if i + 1 < n5:
            Am[i, i + 1] = 1.0
        Am[i, 0] -= A[1:][i]
    Bm = B[1:] - A[1:] * b0
    h = np.zeros(KH)
    h[0] = b0
    z = Bm.copy()
    for t in range(1, KH):
        h[t] = z[0]
        z = Am @ z
    g = np.zeros(KH)
    z = ZI.copy()
    for t in range(KH):
        g[t] = z[0]
        z = Am @ z

    def _hann(m):
        return 0.5 - 0.5 * np.cos(2.0 * np.pi * np.arange(m) / m)

    FW = _hann(W)
    ov = -(-W // HOP)
    den = np.pad(FW ** 2, (0, ov * HOP - W)).reshape(ov, HOP).sum(0)
    den = np.tile(den, ov)[:W]
    SYN = FW / den

    Ire = np.fft.irfft(np.eye(512), n=W, axis=-1)
    Iim = np.fft.irfft(1j * np.eye(512), n=W, axis=-1)
    W_ir = np.concatenate([Ire, Iim], 0) * SYN[None, :]          # [1024, 1022]
    Rf = np.fft.rfft(np.diag(FW), axis=-1)                       # [1022, 512]
    W_f = np.concatenate([np.real(Rf), np.imag(Rf)], 1)          # [1022, 1024]
    W_f = np.pad(W_f, ((0, 2), (0, 0)))                          # [1024, 1024]

    idx = np.arange(128)
    D0 = idx[None, :] - idx[:, None]

    def hmat(args):
        m = np.zeros((128, 128))
        ok = (args >= 0) & (args < KH)
        m[ok] = h[args[ok]]
        return m

    toep = np.stack([hmat(D0), hmat(D0 + 128), hmat(-D0), hmat(-D0 + 128)])

    # edge matrices, K=128 lhsT; rows are absolute partitions of the rhs column
    edges = np.zeros((128, 384))
    for j in range(18):
        # left pad: partitions 110+j of b0; reads x[0..18] in b1 at rows 0..18
        edges[0, 110 + j] += 2.0
        edges[18 - j, 110 + j] -= 1.0
    for j in range(2):
        # right pad head: partitions 126+j of b134; reads x[T-19..T-1] at rows 107..125
        edges[125, 128 + 126 + j] += 2.0
        edges[107 + 17 - j, 128 + 126 + j] -= 1.0
    for j in range(2, 18):
        # right pad tail: partitions j-2 of b135
        edges[125, 256 + j - 2] += 2.0
        edges[107 + 17 - j, 256 + j - 2] -= 1.0

    gmat = np.zeros((128, 384))
    gmat[110, 110:128] = g[0:18]                     # fwd b=0: reads xe0 at partition 110
    gmat[110, 128:238] = g[18:128]                   # fwd b=1
    jj = np.arange(128)
    gi = 143 - jj
    ok = (gi >= 0) & (gi < KH)
    gr = np.zeros(128)
    gr[ok] = g[gi[ok]]
    gmat[15, 256:384] = gr                           # bwd b=134: reads y1 at partition 15

    import ml_dtypes
    f32 = np.float32
    bf16 = ml_dtypes.bfloat16

    def split(a):
        a = a.astype(f32)
        hi = a.astype(bf16)
        lo = (a - hi.astype(f32)).astype(bf16)
        return np.ascontiguousarray(hi), np.ascontiguousarray(lo)

    gmat_h, gmat_l = split(gmat)
    edges_h = np.ascontiguousarray(edges.astype(f32).astype(bf16))  # +-1,+-2: exact
    return dict(
        w_ir=np.ascontiguousarray(W_ir, dtype=f32),
        w_f=np.ascontiguousarray(W_f, dtype=f32),
        toep=np.ascontiguousarray(toep, dtype=f32), edges_h=edges_h,
        gmat_h=gmat_h, gmat_l=gmat_l,
    )


# ---------------------------------------------------------------- bass program
_CACHE = {}


def _build_program():
    import concourse.mybir as mybir
    from concourse.bacc import Bacc
    from concourse.tile import TileContext

    f32 = mybir.dt.float32
    f32r = mybir.dt.float32r
    bf = mybir.dt.bfloat16

    nc = Bacc()
    x = nc.dram_tensor("x", [S, 512, 64, 2], f32, kind="ExternalInput")
    dr = {}
    dr["w_ir"] = nc.dram_tensor("w_ir", [1024, 1022], f32, kind="ExternalInput")
    dr["w_f"] = nc.dram_tensor("w_f", [1024, 1024], f32, kind="ExternalInput")
    dr["toep"] = nc.dram_tensor("toep", [4, 128, 128], f32, kind="ExternalInput")
    for nm, shp in [
        ("edges_h", [128, 384]), ("gmat_h", [128, 384]), ("gmat_l", [128, 384]),
    ]:
        dr[nm] = nc.dram_tensor(nm, shp, bf, kind="ExternalInput")
    out = nc.dram_tensor("out", [S, 512, 64, 2], f32, kind="ExternalOutput")

    with TileContext(nc) as tc:
        with (
            tc.tile_pool(name="const", bufs=1) as cpool,
            tc.tile_pool(name="work", bufs=1) as wpool,
            tc.tile_pool(name="psum", bufs=4, space="PSUM") as ppool,
            tc.tile_pool(name="psum_s", bufs=1, space="PSUM") as pspool,
        ):
            # ---- input first: its consumers gate everything downstream
            xin = wpool.tile([128, 4, S, 128], f32r, tag="xin")
            xre = x[:].rearrange("s (ki p) f c -> p ki s (f c)", p=128).bitcast(f32r)
            for ki in range(4):
                nc.sync.dma_start(out=xin[:, ki], in_=xre[:, ki])

            # ---- constant loads, chunked so the first matmuls start early
            def cload(nm, shape, rearr=None):
                t = cpool.tile(shape, bf, tag=nm)
                src_ap = dr[nm][:]
                if rearr:
                    src_ap = src_ap.rearrange(*rearr[0], **rearr[1])
                nc.sync.dma_start(out=t[:], in_=src_ap)
                return t

            wir = cpool.tile([128, 8, 1022], f32r, tag="w_ir")
            wh_src = dr["w_ir"][:].rearrange("(r p) n -> p r n", p=128).bitcast(f32r)
            for r in range(8):
                nc.sync.dma_start(out=wir[:, r], in_=wh_src[:, r])
            toep = cpool.tile([128, 4, 128], f32r, tag="toep")
            nc.sync.dma_start(
                out=toep[:],
                in_=dr["toep"][:].rearrange("i p n -> p i n").bitcast(f32r))
            edgesh = cload("edges_h", [128, 384])
            gmath = cload("gmat_h", [128, 384])
            gmatl = cload("gmat_l", [128, 384])
            wf = cpool.tile([128, 8, 1024], f32r, tag="w_f")
            fh_src = dr["w_f"][:].rearrange("(j p) n -> p j n", p=128).bitcast(f32r)
            for j in range(8):
                nc.sync.dma_start(out=wf[:, j], in_=fh_src[:, j])

            def ladder(ps_ap, lh, ll, rh, rl, first, last):
                nc.tensor.matmul(ps_ap, lh, rh, start=first, stop=False)
                nc.tensor.matmul(ps_ap, lh, rl, start=False, stop=False)
                nc.tensor.matmul(ps_ap, ll, rh, start=False, stop=last)

            U = wpool.tile([128, S, NBU], f32, tag="U")
            nc.gpsimd.memset(U[:], 0.0)

            # ---- iSTFT: 8 M-chunks x 8 K-chunks x ladder, strided OLA adds
            for m in range(8):
                M = 126 if m == 7 else 128
                ps = ppool.tile([128, S, 64], f32, tag="ps")
                for r in range(8):
                    c, ki = r // 4, r % 4
                    nc.tensor.matmul(ps[:M], wir[:, r, 128 * m:128 * m + M],
                                     xin[:, ki, :, c::2],
                                     start=(r == 0), stop=(r == 7))
                nc.vector.tensor_add(
                    out=U[:M, :, m + 2:m + 2 + 128:2],
                    in0=U[:M, :, m + 2:m + 2 + 128:2],
                    in1=ps[:M],
                )

            Ur = wpool.tile([128, S, NBU], f32r, tag="Ur")

            # ---- odd-extension pads from the raw signal (cols 2 and 135)
            ps_e = pspool.tile([128, S, 3], f32, tag="pse")
            ehL = edgesh[:, 0:128]
            ehR1 = edgesh[:, 128:256]
            ehR2 = edgesh[:, 256:384]
            # split only the two source columns first
            uh2 = wpool.tile([128, S, 2], bf, tag="uh2")
            ul2 = wpool.tile([128, S, 2], bf, tag="ul2")
            for i, col in enumerate((2, 135)):
                nc.vector.tensor_copy(out=uh2[:, :, i], in_=U[:, :, col])
                nc.vector.tensor_sub(out=ul2[:, :, i], in0=U[:, :, col], in1=uh2[:, :, i])
            for (dst, lh, coli) in ((0, ehL, 0), (1, ehR1, 1), (2, ehR2, 1)):
                nc.tensor.matmul(ps_e[:, :, dst:dst + 1], lh, uh2[:, :, coli:coli + 1],
                                 start=True, stop=False)
                nc.tensor.matmul(ps_e[:, :, dst:dst + 1], lh, ul2[:, :, coli:coli + 1],
                                 start=False, stop=True)
            nc.vector.tensor_add(out=U[:, :, 1:2], in0=U[:, :, 1:2], in1=ps_e[:, :, 0:1])
            nc.vector.tensor_add(out=U[:, :, 135:137], in0=U[:, :, 135:137],
                                 in1=ps_e[:, :, 1:3])

            # now mirror the padded signal as f32r for the conv matmuls
            nc.vector.tensor_copy(out=Ur[:], in_=U[:])

            Y1 = wpool.tile([128, S, NBU], f32, tag="Y1")
            nc.gpsimd.memset(Y1[:], 0.0)

            # ---- forward FIR pass (causal), blocks b0..b135
            for (b0, nb) in ((0, 64), (64, 64), (128, 8)):
                ps = ppool.tile([128, S, 64], f32, tag="ps")
                nc.tensor.matmul(ps[:, :, :nb], toep[:, 0, :],
                                 Ur[:, :, 1 + b0:1 + b0 + nb], start=True, stop=False)
                nc.tensor.matmul(ps[:, :, :nb], toep[:, 1, :],
                                 Ur[:, :, b0:b0 + nb], start=False, stop=True)
                if b0 == 128:
                    nc.vector.tensor_copy(out=Y1[:, :, 128:135], in_=ps[:, :, 0:7])
                    nc.vector.tensor_copy(out=Y1[0:16, :, 135], in_=ps[0:16, :, 7])
                else:
                    nc.vector.tensor_copy(out=Y1[:, :, b0:b0 + nb], in_=ps[:, :, :nb])

            # zi-seeding correction at the left edge (rank-1); rhs is U col 1
            # (xe0 at partition 110), split on the fly
            ps_g = pspool.tile([128, S, 2], f32, tag="psg")
            uh1 = wpool.tile([128, S, 2], bf, tag="uh1")
            ul1 = wpool.tile([128, S, 2], bf, tag="ul1")
            nc.vector.tensor_copy(out=uh1[:, :, 0], in_=U[:, :, 1])
            nc.vector.tensor_sub(out=ul1[:, :, 0], in0=U[:, :, 1], in1=uh1[:, :, 0])
            for (dst, cl) in ((0, slice(0, 128)), (1, slice(128, 256))):
                ladder(ps_g[:, :, dst:dst + 1], gmath[:, cl], gmatl[:, cl],
                       uh1[:, :, 0:1], ul1[:, :, 0:1], True, True)
            nc.vector.tensor_add(out=Y1[:, :, 0:2], in0=Y1[:, :, 0:2],
                                 in1=ps_g[:, :, 0:2])

            Y1r = wpool.tile([128, S, NBU], f32r, tag="Y1r")
            nc.vector.tensor_copy(out=Y1r[:], in_=Y1[:])
            y1h5 = wpool.tile([128, S, 1], bf, tag="y1h5")
            y1l5 = wpool.tile([128, S, 1], bf, tag="y1l5")
            nc.vector.tensor_copy(out=y1h5[:, :, 0], in_=Y1[:, :, 135])
            nc.vector.tensor_sub(out=y1l5[:, :, 0], in0=Y1[:, :, 135], in1=y1h5[:, :, 0])

            Y2 = wpool.tile([128, S, NBU], f32, tag="Y2")

            # ---- backward FIR pass (anti-causal), blocks b1..b134
            for (b0, nb) in ((1, 64), (65, 64), (129, 6)):
                ps = ppool.tile([128, S, 64], f32, tag="ps")
                nc.tensor.matmul(ps[:, :, :nb], toep[:, 2, :],
                                 Y1r[:, :, b0:b0 + nb], start=True, stop=False)
                nc.tensor.matmul(ps[:, :, :nb], toep[:, 3, :],
                                 Y1r[:, :, b0 + 1:b0 + 1 + nb], start=False, stop=True)
                nc.vector.tensor_copy(out=Y2[:, :, b0:b0 + nb], in_=ps[:, :, :nb])

            # zi-seeding correction at the right edge (rank-1)
            ps_g2 = pspool.tile([128, S, 2], f32, tag="psg")
            ladder(ps_g2[:, :, 0:1], gmath[:, 256:384], gmatl[:, 256:384],
                   y1h5[:, :, 0:1], y1l5[:, :, 0:1], True, True)
            nc.vector.tensor_add(out=Y2[:, :, 134:135], in0=Y2[:, :, 134:135],
                                 in1=ps_g2[:, :, 0:1])

            Y2r = wpool.tile([128, S, NBU], f32r, tag="Y2r")
            nc.vector.tensor_copy(out=Y2r[:], in_=Y2[:])

            # ---- forward STFT: 8 M-chunks x 8 frame-chunks x ladder
            outsb = wpool.tile([128, 4, S, 128], f32, tag="osb")
            orr = out[:].rearrange("s (ki p) f c -> p ki s (f c)", p=128)
            # pair the re/im chunks of each output k-block so its store DMA
            # drains while later chunks are still on the PE
            for m in (0, 4, 1, 5, 2, 6, 3, 7):
                ps = ppool.tile([128, S, 64], f32, tag="ps")
                for j in range(8):
                    nc.tensor.matmul(ps[:], wf[:, j, 128 * m:128 * m + 128],
                                     Y2r[:, :, j + 1:j + 1 + 128:2],
                                     start=(j == 0), stop=(j == 7))
                c, ki = m // 4, m % 4
                nc.vector.tensor_copy(out=outsb[:, ki, :, c::2], in_=ps[:])
                if c == 1:
                    nc.sync.dma_start(out=orr[:, ki], in_=outsb[:, ki])

    nc.compile()
    return nc


def _get_ctx():
    if "nc" not in _CACHE:
        _CACHE["consts"] = _build_consts()
        _CACHE["nc"] = _build_program()
    return _CACHE["nc"], _CACHE["consts"]


def kernel(x: np.ndarray) -> np.ndarray:
    from concourse.bass_utils import run_bass_kernel_spmd

    nc, consts = _get_ctx()
    x = np.ascontiguousarray(x, dtype=np.float32)
    in_maps = []
    for c in range(8):
        m = {"x": np.ascontiguousarray(x[S * c:S * c + S])}
        m.update(consts)
        in_maps.append(m)
    res = run_bass_kernel_spmd(nc, in_maps, core_ids=list(range(8)))
    return np.concatenate([r["out"] for r in res.results], axis=0)



# revision 24
# speedup vs baseline: 1.5202x; 1.5202x over previous
"""Trainium2 Bass kernel: ISTFT -> Butterworth filtfilt -> STFT (LowpassFilter).

v4: time-symmetry folded FFT stages + fused single-pass filtfilt.
  - irfft/rfft matrices are even/odd symmetric about t <-> 1022-t (the Hann
    window and the OLA denominator are both symmetric), so each FFT stage
    contracts only t=0..511: half the matmul instructions and half the
    constant bytes of the dense form.  Mirror halves are recovered with
    cheap 128x128 flip matmuls on the PE (partition reversal is impossible
    on the vector engines).
  - the two FIR passes of filtfilt fuse into one 3-diagonal block-Toeplitz
    pass (A1=T1@T2, A2=T0@T2+T1@T3, A3=T0@T3), bit-equivalent to the 2-pass
    form, with rank-1 zi corrections mapped through the backward pass.
  - big matmuls run in bf16 (constants + input quantized, ~4e-3 rel overall);
    the FIR path stays float32r.
  - input/output DRAM tensors hold the SBUF tile layout (pre-packed on the
    host, unpacked after) so every DMA moves large contiguous runs.
  - PE warmup matmuls during the initial DMA window keep the tensor engine's
    p-state ramp warm so real matmuls run at full clock from the start.
Data parallel: batch 64 -> 8 samples per NeuronCore, SPMD on 8 cores.
"""

import numpy as np

W = 1022
HOP = 256
F = 64
ORDER = 5
WN = 0.5
T = HOP * (F - 1) + W  # 17150
KH = 128               # FIR truncation of the IIR impulse response
NBU = 137              # signal blocks per channel in U: [lookback, b0..b135]
S = 8                  # samples (channels) per core


# ---------------------------------------------------------------- constants
def _butter_lowpass(order, wn):
    m = np.arange(-order + 1, order, 2)
    p = -np.exp(1j * np.pi * m / (2 * order))
    fs = 2.0
    warped = 2 * fs * np.tan(np.pi * wn / fs)
    p = p * warped
    k = warped ** order
    fs2 = 2 * fs
    pd = (fs2 + p) / (fs2 - p)
    kd = k * np.real(1.0 / np.prod(fs2 - p))
    b = np.real(kd * np.poly(-np.ones(order)))
    a = np.real(np.poly(pd))
    return b / a[0], a / a[0]


def _build_consts():
    import ml_dtypes

    f32 = np.float32
    bf16 = ml_dtypes.bfloat16

    B, A = _butter_lowpass(ORDER, WN)
    n = max(len(A), len(B))
    Am0 = np.zeros((n - 1, n - 1))
    Am0[0, :] = -A[1:]
    Am0[1:, :-1] = np.eye(n - 2)
    Am0 = Am0.T
    ZI = np.linalg.solve(np.eye(n - 1) - Am0, B[1:] - A[1:] * B[0])

    b0 = B[0]
    n5 = 5
    Am = np.zeros((n5, n5))
    for i in range(n5):
        if i + 1 < n5:
            Am[i, i + 1] = 1.0
        Am[i, 0] -= A[1:][i]
    Bm = B[1:] - A[1:] * b0
    h = np.zeros(KH)
    h[0] = b0
    z = Bm.copy()
    for t in range(1, KH):
        h[t] = z[0]
        z = Am @ z
    g = np.zeros(KH)
    z = ZI.copy()
    for t in range(KH):
        g[t] = z[0]
        z = Am @ z

    def _hann(m):
        return 0.5 - 0.5 * np.cos(2.0 * np.pi * np.arange(m) / m)

    FW = _hann(W)
    ov = -(-W // HOP)
    den = np.pad(FW ** 2, (0, ov * HOP - W)).reshape(ov, HOP).sum(0)
    den = np.tile(den, ov)[:W]
    SYN = FW / den

    # folded iSTFT matrices: [512 bins, 512 t], t = 0..511
    Ire = np.fft.irfft(np.eye(512), n=W, axis=-1)
    Iim = np.fft.irfft(1j * np.eye(512), n=W, axis=-1)
    WirC = (Ire * SYN[None, :])[:, :512]
    WirS = (Iim * SYN[None, :])[:, :512]
    wirc = np.transpose(WirC.reshape(4, 128, 512), (1, 0, 2))  # [p, kc, t]
    wirs = np.transpose(WirS.reshape(4, 128, 512), (1, 0, 2))

    # folded STFT matrices: [512 t, 512 bins]
    theta = 2 * np.pi / W
    kk = np.arange(512)
    tcol = np.arange(512)
    CcosF = FW[tcol, None] * np.cos(theta * np.outer(tcol, kk))
    CsinF = -FW[tcol, None] * np.sin(theta * np.outer(tcol, kk))
    CcosF[511] /= 2.0  # t=511 is self-paired in the fold
    wfc = np.transpose(CcosF.reshape(4, 128, 512), (1, 0, 2))  # [p, j, k]
    wfs = np.transpose(CsinF.reshape(4, 128, 512), (1, 0, 2))

    # flip matrices (lhsT layout [K=p_in, M=p_out]):
    #   Jf[p, q] = 1 iff q == 126 - p ;  E127[p, q] = 1 iff p == q == 127
    Jf = np.zeros((128, 128))
    Jf[np.arange(127), 126 - np.arange(127)] = 1.0
    E127 = np.zeros((128, 128))
    E127[127, 127] = 1.0
    jmat = np.transpose(np.stack([Jf, E127], 0), (1, 0, 2))  # [p, 2, q]

    # column-flipped iSTFT matrices for the mirror half: output chunk c
    # (c=4..7), out partition q ->  t = 1022 - 128c - q  (0 column if the
    # output time 128c+q+512 exceeds 1021, i.e. c=7, q>=126)
    WirCF = np.zeros((512, 512))
    WirSF = np.zeros((512, 512))
    for c in range(4, 8):
        for q in range(128):
            tt_ = 1022 - 128 * c - q
            if 0 <= tt_ <= 511 and 128 * c + q <= 1021:
                WirCF[:, 128 * (c - 4) + q] = WirC[:, tt_]
                WirSF[:, 128 * (c - 4) + q] = WirS[:, tt_]
    wircf = np.transpose(WirCF.reshape(4, 128, 512), (1, 0, 2))
    wirsf = np.transpose(WirSF.reshape(4, 128, 512), (1, 0, 2))

    # fused filtfilt block-Toeplitz diagonals
    idx = np.arange(128)
    D0 = idx[None, :] - idx[:, None]

    def hmat(args):
        m = np.zeros((128, 128))
        ok = (args >= 0) & (args < KH)
        m[ok] = h[args[ok]]
        return m

    T0, T1, T2, T3 = hmat(D0), hmat(D0 + 128), hmat(-D0), hmat(-D0 + 128)
    afir = np.stack([T1 @ T2, T0 @ T2 + T1 @ T3, T0 @ T3], 0)  # [3, 128, 128]
    afir = np.transpose(afir, (1, 0, 2))                        # [p, 3, q]

    # y1_end extraction: y1_end = T1[:,15].U[135] + T0[:,15].U[136]
    zmat = np.zeros((128, 2, 128))
    zmat[:, 0, 15] = T1[:, 15]
    zmat[:, 1, 15] = T0[:, 15]


    # edge matrices (odd-extension pads), K=128 lhsT
    edges = np.zeros((128, 384))
    for j in range(18):
        edges[0, 110 + j] += 2.0
        edges[18 - j, 110 + j] -= 1.0
    for j in range(2):
        edges[125, 128 + 126 + j] += 2.0
        edges[107 + 17 - j, 128 + 126 + j] -= 1.0
    for j in range(2, 18):
        edges[125, 256 + j - 2] += 2.0
        edges[107 + 17 - j, 256 + j - 2] -= 1.0

    # zi corrections: left mapped through the bwd pass (w1), right (gr)
    gmatv1 = np.zeros(128)
    gmatv1[:110] = g[18:128]
    w1 = gmatv1 @ T2
    jj = np.arange(128)
    gi = 143 - jj
    ok = (gi >= 0) & (gi < KH)
    gr = np.zeros(128)
    gr[ok] = g[gi[ok]]
    gmat2 = np.zeros((128, 256))
    gmat2[110, 0:128] = w1
    gmat2[15, 128:256] = gr

    return dict(
        wirc=np.ascontiguousarray(wirc.astype(f32).astype(bf16)),
        wirs=np.ascontiguousarray(wirs.astype(f32).astype(bf16)),
        wfc=np.ascontiguousarray(wfc.astype(f32).astype(bf16)),
        wfs=np.ascontiguousarray(wfs.astype(f32).astype(bf16)),
        wircf=np.ascontiguousarray(wircf.astype(f32).astype(bf16)),
        wirsf=np.ascontiguousarray(wirsf.astype(f32).astype(bf16)),
        jmat=np.ascontiguousarray(jmat.astype(f32).astype(bf16)),
        afir=np.ascontiguousarray(afir.astype(f32).astype(bf16)),
        zmat=np.ascontiguousarray(zmat.astype(f32).astype(bf16)),
        edges_h=np.ascontiguousarray(edges.astype(f32).astype(bf16)),
        gmat2=np.ascontiguousarray(gmat2.astype(f32).astype(bf16)),
    )


# ---------------------------------------------------------------- bass program
_CACHE = {}


def _build_program():
    import concourse.mybir as mybir
    from concourse.bacc import Bacc
    from concourse.tile import TileContext

    f32 = mybir.dt.float32
    f32r = mybir.dt.float32r
    bf = mybir.dt.bfloat16

    nc = Bacc()
    # input pre-packed on host: xin[p, ki, s, 2f+c] = x[s, 128ki+p, f, c], bf16
    x = nc.dram_tensor("x", [128, 4, S, 128], bf, kind="ExternalInput")
    dr = {}
    for nm, shp, dt in [
        ("wirc", [128, 4, 512], bf), ("wirs", [128, 4, 512], bf),
        ("wfc", [128, 4, 512], bf), ("wfs", [128, 4, 512], bf),
        ("wircf", [128, 4, 512], bf), ("wirsf", [128, 4, 512], bf),
        ("jmat", [128, 2, 128], bf), ("afir", [128, 3, 128], bf),
        ("zmat", [128, 2, 128], bf), ("edges_h", [128, 384], bf),
        ("gmat2", [128, 256], bf),
    ]:
        dr[nm] = nc.dram_tensor(nm, shp, dt, kind="ExternalInput")
    out = nc.dram_tensor("out", [128, 4, S, 128], bf, kind="ExternalOutput")

    with TileContext(nc) as tc:
        with (
            tc.tile_pool(name="const", bufs=1) as cpool,
            tc.tile_pool(name="work", bufs=1) as wpool,
            tc.tile_pool(name="psum", bufs=4, space="PSUM") as ppool,
        ):
            # single uniform psum tag: 4 slots x 4KB.  OLA accumulators are
            # flat-reinterpreted views of two slots; FIFO slot rotation is
            # matched to the free order of earlier tiles.
            _pb = [0]

            def bigp():
                _pb[0] += 1
                return ppool.tile([128, 2, S, F], f32, tag="big",
                                  name=f"big{_pb[0]}")

            def accview(tile, n):
                flat = tile[:].rearrange("p i s f -> p (i s f)")
                return flat[:, 0:8 * n].rearrange("p (a b) -> p a b", b=8)

            # ---- PE warmup: dummy matmuls bridge the initial DMA window so
            # the p-state ramp is fully warm when real matmuls arrive.
            wu = wpool.tile([128, 512], bf, tag="wu")
            nc.gpsimd.memset(wu[:], 0.0)
            pw = bigp()
            for _ in range(6):
                nc.tensor.matmul(pw[:, 0], wu[:, 0:128], wu[:],
                                 start=True, stop=True, skip_group_check=True)

            upet = bigp()
            upot = bigp()
            upe = accview(upet, 80)   # even cols 2..136 at slots 0..67
            upo = accview(upot, 80)   # odd cols 3..135 at slots 0..66
            fir12 = bigp()

            # ---- DMA order == first-use order (each DMA pays ~900ns of
            # semaphore propagation, so the front streams interleave per kc).
            wirc = cpool.tile([128, 4, 512], bf, tag="wirc")
            wirs = cpool.tile([128, 4, 512], bf, tag="wirs")
            xin = wpool.tile([128, 4, S, 128], bf, tag="xin")
            for kc in range(4):
                nc.sync.dma_start(out=wirc[:, kc], in_=dr["wirc"][:, kc])
                nc.sync.dma_start(out=xin[:, kc], in_=x[:, kc])
                nc.sync.dma_start(out=wirs[:, kc], in_=dr["wirs"][:, kc])
            wircf = cpool.tile([128, 4, 512], bf, tag="wircf")
            wirsf = cpool.tile([128, 4, 512], bf, tag="wirsf")
            for kc in range(4):
                nc.sync.dma_start(out=wircf[:, kc], in_=dr["wircf"][:, kc])
                nc.sync.dma_start(out=wirsf[:, kc], in_=dr["wirsf"][:, kc])
            edgesh = cpool.tile([128, 384], bf, tag="edges_h")
            nc.sync.dma_start(out=edgesh[:], in_=dr["edges_h"][:])
            gmat2 = cpool.tile([128, 256], bf, tag="gmat2")
            nc.sync.dma_start(out=gmat2[:], in_=dr["gmat2"][:])
            zmat = cpool.tile([128, 2, 128], bf, tag="zmat")
            nc.sync.dma_start(out=zmat[:], in_=dr["zmat"][:])
            afir = cpool.tile([128, 3, 128], bf, tag="afir")
            nc.sync.dma_start(out=afir[:], in_=dr["afir"][:])
            jmatb = cpool.tile([128, 2, 128], bf, tag="jmat")
            nc.sync.dma_start(out=jmatb[:], in_=dr["jmat"][:])
            wfc = cpool.tile([128, 4, 512], bf, tag="wfc")
            nc.sync.dma_start(out=wfc[:], in_=dr["wfc"][:])
            wfs = cpool.tile([128, 4, 512], bf, tag="wfs")
            nc.sync.dma_start(out=wfs[:], in_=dr["wfs"][:])

            # negated imag rhs for the mirror-half sin matmuls
            xinN = wpool.tile([128, 4, S, F], bf, tag="xinN")
            for kc in range(4):
                nc.vector.tensor_scalar_mul(out=xinN[:, kc],
                                            in0=xin[:, kc, :, 1::2],
                                            scalar1=-1.0)

            U = wpool.tile([128, S, NBU], bf, tag="U")
            nc.vector.memset(U[:, :, 0:2], 0.0)
            # accumulator fringe slots (64..67) take start=False writes from
            # the shifted windows; zero them first
            nc.vector.memset(upe[:, 64:68], 0.0)
            nc.vector.memset(upo[:, 64:68], 0.0)

            # ---- folded iSTFT with the whole OLA accumulated in PSUM.
            # U column c = m+2+2f (A-half) / (c-4)+6+2f (mirror half via
            # column-flipped constants).  Parity = col%2, slot offsets 0..3.
            def accv(par, off):
                acc = upe if par == 0 else upo
                return acc[:, off:off + 64].rearrange("p f s -> p s f")

            for kc in range(4):
                for m in range(4):
                    nc.tensor.matmul(accv(m % 2, m // 2),
                                     wirc[:, kc, 128 * m:128 * m + 128],
                                     xin[:, kc, :, 0::2],
                                     start=(kc == 0 and m < 2), stop=False,
                                     skip_group_check=True)
                for m in range(4):
                    nc.tensor.matmul(accv(m % 2, m // 2),
                                     wirs[:, kc, 128 * m:128 * m + 128],
                                     xin[:, kc, :, 1::2],
                                     start=False, stop=False,
                                     skip_group_check=True)
            for kc in range(4):
                for c in range(4, 8):
                    nc.tensor.matmul(accv(c % 2, 2 + (c - 4) // 2),
                                     wircf[:, kc, 128 * (c - 4):128 * (c - 3)],
                                     xin[:, kc, :, 0::2],
                                     start=False, stop=False,
                                     skip_group_check=True)
                for c in range(4, 8):
                    nc.tensor.matmul(accv(c % 2, 2 + (c - 4) // 2),
                                     wirsf[:, kc, 128 * (c - 4):128 * (c - 3)],
                                     xinN[:, kc],
                                     start=False,
                                     stop=(kc == 3 and c >= 6),
                                     skip_group_check=True)

            # psum -> U in parallel on Act (even) and DVE (odd)
            nc.scalar.copy(
                out=U[:, :, 2:137:2],
                in_=upe[:, 0:68].rearrange("p f s -> p s f"))
            nc.vector.tensor_copy(
                out=U[:, :, 3:137:2],
                in_=upo[:, 0:67].rearrange("p f s -> p s f"))

            # small edge/zi results live in the accumulator tails: early-
            # consumed ones in the even tile (freed first), zi ones in odd
            egv = upe[:, 68:80].rearrange("p f s -> p s f")   # [128, 8, 12]
            ogv = upo[:, 68:80].rearrange("p f s -> p s f")
            ps_e = egv[:, :, 0:1]
            ps_y = egv[:, :, 3:4]
            ps_r = egv[:, :, 4:6]
            ps_g = ogv[:, :, 1:2]
            ps_g2 = ogv[:, :, 2:3]

            # ---- odd-extension pads + zi rhs (all-bf16; U is bf16 so its
            # column slices serve directly as matmul rhs; program order
            # sequences the read-before-pad-add hazards)
            nc.tensor.matmul(ps_e, edgesh[:, 0:128], U[:, :, 2:3],
                             start=True, stop=True, skip_group_check=True)
            nc.vector.tensor_add(out=U[:, :, 1:2], in0=U[:, :, 1:2], in1=ps_e)
            nc.tensor.matmul(ps_g, gmat2[:, 0:128], U[:, :, 1:2],
                             start=True, stop=True, skip_group_check=True)
            for (dst, lh) in ((0, edgesh[:, 128:256]), (1, edgesh[:, 256:384])):
                nc.tensor.matmul(ps_r[:, :, dst:dst + 1], lh, U[:, :, 135:136],
                                 start=True, stop=True, skip_group_check=True)
            nc.vector.tensor_add(out=U[:, :, 135:137], in0=U[:, :, 135:137],
                                 in1=ps_r)
            nc.tensor.matmul(ps_y, zmat[:, 0], U[:, :, 135:136],
                             start=True, stop=False, skip_group_check=True)
            nc.tensor.matmul(ps_y, zmat[:, 1], U[:, :, 136:137],
                             start=False, stop=True, skip_group_check=True)
            yec = wpool.tile([128, S, 1], bf, tag="yec")
            nc.vector.tensor_copy(out=yec[:, :, 0], in_=ps_y[:, :, 0])
            nc.tensor.matmul(ps_g2, gmat2[:, 128:256], yec[:, :, 0:1],
                             start=True, stop=True, skip_group_check=True)

            Y2 = wpool.tile([128, S, NBU], bf, tag="Y2")

            # ---- fused filtfilt: Y2[b] = A1.U[b] + A2.U[b+1] + A3.U[b+2]
            for gi, b0 in enumerate((1, 65)):
                for d in range(3):
                    nc.tensor.matmul(fir12[:, gi], afir[:, d, :],
                                     U[:, :, b0 + d:b0 + d + 64],
                                     start=(d == 0), stop=(d == 2))
            nc.scalar.copy(
                out=Y2[:, :, 1:129].rearrange("p s (i f) -> p i s f", i=2),
                in_=fir12[:])
            fir3t = bigp()
            fir3 = fir3t[:, 0, :, 0:6]
            for d in range(3):
                nc.tensor.matmul(fir3, afir[:, d, :],
                                 U[:, :, 129 + d:135 + d],
                                 start=(d == 0), stop=(d == 2))
            nc.vector.tensor_copy(out=Y2[:, :, 129:135], in_=fir3)
            # zi corrections into Y2 cols 1 and 134 (col 134 only gates the
            # j=0 mirror, handled in the second y2m pair)
            nc.vector.tensor_add(out=Y2[:, :, 1:2], in0=Y2[:, :, 1:2],
                                 in1=ps_g)
            nc.vector.tensor_add(out=Y2[:, :, 134:135], in0=Y2[:, :, 134:135],
                                 in1=ps_g2)

            # ---- STFT fold prep: pair (2,3) reads only cols <=133 so it
            # runs before the zi adds land; pair (0,1) last.
            # y2m_j[q, s, f] = Y2[126-q, s, 8-j+2f] + E127 shift
            uu = wpool.tile([128, 4, S, F], bf, tag="uu")
            vv = wpool.tile([128, 4, S, F], bf, tag="vv")
            JORD = (2, 3, 0, 1)
            yms = {}
            prp = {}
            for pi_, j0 in enumerate((2, 0)):
                ym = bigp()
                yms[j0] = ym
                for i in range(2):
                    j = j0 + i
                    nc.tensor.matmul(ym[:, i], jmatb[:, 0],
                                     Y2[:, :, 8 - j:8 - j + 128:2],
                                     start=True, stop=False)
                    nc.tensor.matmul(ym[:, i], jmatb[:, 1],
                                     Y2[:, :, 7 - j:7 - j + 128:2],
                                     start=False, stop=True)
                prp[pi_] = bigp()
                ap = Y2[:, :, j0 + 1:j0 + 1 + 128].rearrange(
                    "p s (f i) -> p i s f", i=2)
                nc.vector.tensor_add(out=uu[:, j0:j0 + 2], in0=ap, in1=ym[:])

            for j0 in (2, 0):
                ap = Y2[:, :, j0 + 1:j0 + 1 + 128].rearrange(
                    "p s (f i) -> p i s f", i=2)
                nc.vector.tensor_sub(out=vv[:, j0:j0 + 2], in0=ap,
                                     in1=yms[j0][:])

            # ---- folded STFT: re j-outer over both m-pair tiles (starts on
            # u23), then im m-outer pipelining the output copies + stores.
            outsb = wpool.tile([128, 4, S, 128], bf, tag="osb")
            for ji, j in enumerate(JORD):
                for m in range(4):
                    nc.tensor.matmul(prp[m // 2][:, m % 2],
                                     wfc[:, j, 128 * m:128 * m + 128],
                                     uu[:, j], start=(ji == 0), stop=(ji == 3))
            for mp in range(2):
                nc.scalar.copy(out=outsb[:, 2 * mp:2 * mp + 2, :, 0::2],
                               in_=prp[mp][:])
            for mp in range(2):
                pi = bigp()
                for mi in range(2):
                    m = 2 * mp + mi
                    for ji, j in enumerate(JORD):
                        nc.tensor.matmul(pi[:, mi],
                                         wfs[:, j, 128 * m:128 * m + 128],
                                         vv[:, j], start=(ji == 0),
                                         stop=(ji == 3))
                    if m % 2 == 0:
                        nc.scalar.copy(out=outsb[:, m, :, 1::2],
                                       in_=pi[:, mi])
                    else:
                        nc.vector.tensor_copy(out=outsb[:, m, :, 1::2],
                                              in_=pi[:, mi])
                    nc.sync.dma_start(out=out[:, m], in_=outsb[:, m])

    nc.compile()
    return nc


def _get_ctx():
    if "nc" not in _CACHE:
        _CACHE["consts"] = _build_consts()
        _CACHE["nc"] = _build_program()
    return _CACHE["nc"], _CACHE["consts"]


def _pack_x(xc):
    """[S, 512, F, 2] f32 -> [128, 4, S, 128] bf16 tile layout."""
    import ml_dtypes

    t = xc.reshape(S, 4, 128, F * 2)
    t = np.transpose(t, (2, 1, 0, 3))
    return np.ascontiguousarray(t.astype(ml_dtypes.bfloat16))


def _unpack_out(o):
    """[128, 4, S, 128] bf16 -> [S, 512, F, 2] f32."""
    t = np.transpose(o.astype(np.float32), (2, 1, 0, 3))
    return np.ascontiguousarray(t.reshape(S, 512, F, 2))


def kernel(x: np.ndarray) -> np.ndarray:
    from concourse.bass_utils import run_bass_kernel_spmd

    nc, consts = _get_ctx()
    x = np.ascontiguousarray(x, dtype=np.float32)
    in_maps = []
    for c in range(8):
        m = {"x": _pack_x(x[S * c:S * c + S])}
        m.update(consts)
        in_maps.append(m)
    res = run_bass_kernel_spmd(nc, in_maps, core_ids=list(range(8)))
    return np.concatenate([_unpack_out(r["out"]) for r in res.results], axis=0)
